# revision 32
# baseline (speedup 1.0000x reference)
"""Trainium2 Bass kernel: dense-CRF mean-field layer (96x96 image, 21 labels).

Strategy (8 NeuronCores, row-sharded):
  * K_bl [N, S-slice] built once on-device in fp16 (fp8 fails the accuracy
    budget: the mean-field dynamics amplify ~3% kernel noise into ~3e-2
    worst-pixel errors) and kept SBUF-resident per core.
  * The K_bl build (d2 matmul + Exp on the scalar engine) is fused with
    iteration-0's bilateral accumulation: chunk matmuls fire as their K_bl
    columns come out of the exp, hiding all of iteration 0's PE work inside
    the ACT-bound exp window.  Optionally a small fraction of exp windows
    run on the vector engine as a 16-bit Schraudolph (u16 bitcast fp16).
  * q carries a 22nd norm channel = 0.125 and labels pre-scaled by 1.25, so
    P_l / P_norm = 10 * message (W_BILATERAL folded); the spatial blur
    matrices absorb the 1/1.25.
  * Spatial kernel is separable: x-blur per core pre-gather, y-blur
    post-gather (W_SPATIAL/norm folded into host blur matrices); the
    x-blurred t1 rides the AllGather in fp8 (accuracy-checked).
  * Per-iteration tail is group-pipelined (3 y-groups: burst -> transpose ->
    softmax -> x-blur while the next group's burst runs), q is repacked to
    chunk-major via PE transposes + a repack matmul (no partition-shift DMA
    dance), and the gathered payloads are reassembled with one merged DMA
    per destination tensor.
"""
import sys
sys.path.insert(0, "/opt/trn_rl_repo")
import os
import numpy as np
import ml_dtypes

H = W = 96
N = H * W                  # 9216
L = 21
LE = L + 1                 # 22 channels (21 labels + norm channel)
ALPHA, BETA, GAMMA = 80.0, 13.0, 3.0
W_SPATIAL, W_BILATERAL = 3.0, 10.0
NUM_ITERATIONS = 5
NCORES = 8
S = N // NCORES            # 1152 rows per core
YPC = H // NCORES          # 12 image rows per core
CH = N // 128              # 72 chunks of 128 rows (global j)
KCOLS = CH * S             # 82944 K_bl columns (fp16: 2B each)
WIN = 1024                 # d2/exp window (81 windows exactly)
NWIN = KCOLS // WIN
PAYQ = 128 * 9 * LE * 2    # 50688 payload bytes (q part, fp16)
PAYT = S * LE              # 25344 payload bytes (t1 part, fp8)
A16 = 1024.0 / float(np.log(2.0))   # fp16 Schraudolph scale (10 mantissa bits)
B16 = float(15 * 1024)              # fp16 exponent bias 15 << 10
ONESV = 0.125              # q norm-channel value
QSCALE = 1.25              # label-channel pre-scale; 1.25/0.125 = W_BILATERAL
DVE_SHARE = float(os.environ.get("CRF_DVE_SHARE", "0.0"))

# Bilateral output groups (by y-rows of this core's slice) for tail pipelining
GROUPS = (
    (0, 5, ((0, 480),)),
    (5, 10, ((480, 32), (512, 448))),
    (10, 12, ((960, 64), (1024, 128))),
)
T2_SLICES = [(0, 512), (512, 512), (1024, 512), (1536, 512), (2048, 64)]

LAST_EXEC_NS = None
_CACHE = {}


def _dve_window(w):
    """Deterministic ACT/DVE interleave for exp windows."""
    return int((w + 1) * DVE_SHARE) != int(w * DVE_SHARE)


def _build_bass(sim1=False):
    """Build the kernel. sim1=True builds a single-core variant where the
    AllGather is replaced by a broadcast DRAM copy (for TimelineSim)."""
    key = "nc_sim1" if sim1 else "nc"
    if key in _CACHE:
        return _CACHE[key]
    import concourse.bass as bass  # noqa: F401
    from concourse import bacc
    import concourse.mybir as mybir
    import concourse.tile as tile

    f32 = mybir.dt.float32
    bf16 = mybir.dt.bfloat16
    fp16 = mybir.dt.float16
    f8 = mybir.dt.float8e4
    u16 = mybir.dt.uint16
    u8 = mybir.dt.uint8
    AF = mybir.ActivationFunctionType
    OP = mybir.AluOpType
    AX = mybir.AxisListType

    nc = bacc.Bacc("TRN2", target_bir_lowering=False, debug=False,
                   num_devices=1 if sim1 else NCORES)

    featL_d = nc.dram_tensor("featL", [21, N], bf16, kind="ExternalInput")
    featR_d = nc.dram_tensor("featR", [21, S], bf16, kind="ExternalInput")
    uxy_d = nc.dram_tensor("uxy", [W, YPC * LE], f32, kind="ExternalInput")
    Ax_d = nc.dram_tensor("Ax", [W, W], bf16, kind="ExternalInput")
    Ay_d = nc.dram_tensor("Ay", [H, YPC], bf16, kind="ExternalInput")
    qsb0_d = nc.dram_tensor("qsb0", [128, CH * LE], fp16, kind="ExternalInput")
    t1f0_d = nc.dram_tensor("t1f0", [H, LE * W], f8, kind="ExternalInput")
    id22_d = nc.dram_tensor("id22", [LE, LE], f32, kind="ExternalInput")
    id96_d = nc.dram_tensor("id96", [W, W], f32, kind="ExternalInput")
    i22h_d = nc.dram_tensor("i22h", [LE, LE], fp16, kind="ExternalInput")
    qout_d = nc.dram_tensor("qout", [S, L], f32, kind="ExternalOutput")
    dbgsel = int(os.environ.get("CRF_DEBUG", "0"))
    dbg = dbgsel > 0
    if dbg:
        dbg_kbl = nc.dram_tensor("dbg_kbl", [128, 4 * S], fp16,
                                 kind="ExternalOutput")
        dbg_pbl = nc.dram_tensor("dbg_pbl", [LE, S], f32, kind="ExternalOutput")
        dbg_t2r = nc.dram_tensor("dbg_t2r", [LE, YPC * W], f32,
                                 kind="ExternalOutput")
        dbg_qy = nc.dram_tensor("dbg_qy", [W, YPC * LE], f32,
                                kind="ExternalOutput")
        dbg_qsb1 = nc.dram_tensor("dbg_qsb1", [128, CH * LE], fp16,
                                  kind="ExternalOutput")
        dbg_t1f1 = nc.dram_tensor("dbg_t1f1", [H, LE * W], f8,
                                  kind="ExternalOutput")

    with tile.TileContext(nc) as tc:
        with (
            tc.tile_pool(name="const", bufs=1) as constp,
            tc.tile_pool(name="kbl", bufs=1) as kblp,
            tc.tile_pool(name="work", bufs=1) as work,
            tc.tile_pool(name="dram", bufs=2, space="DRAM") as dram,
        ):
            Ax = constp.tile([W, W], bf16)
            nc.sync.dma_start(Ax[:], Ax_d[:])
            Ay = constp.tile([H, YPC], bf16)
            nc.sync.dma_start(Ay[:], Ay_d[:])
            uxy = constp.tile([W, YPC * LE], f32)
            nc.sync.dma_start(uxy[:], uxy_d[:])
            uxy3 = uxy[:].rearrange("x (y l) -> x y l", y=YPC, l=LE)
            id22 = constp.tile([LE, LE], f32)
            nc.sync.dma_start(id22[:], id22_d[:])
            id96 = constp.tile([W, W], f32)
            nc.sync.dma_start(id96[:], id96_d[:])
            i22h = constp.tile([LE, LE], fp16)
            nc.sync.dma_start(i22h[:], i22h_d[:])
            featR = constp.tile([21, S], bf16)
            nc.sync.dma_start(featR[:], featR_d[:])
            Kbl = kblp.tile([128, KCOLS], fp16)
            Kbl_u16 = Kbl[:].bitcast(u16)

            qsb = work.tile([128, CH * LE], fp16, tag="qsb")
            t1full = work.tile([H, LE * W], f8, tag="t1full")
            nc.sync.dma_start(qsb[:], qsb0_d[:])
            nc.scalar.dma_start(t1full[:], t1f0_d[:])

            # ------- precompute K_bl (deep-pipelined over all 8 banks) ------
            with (
                tc.tile_pool(name="pre_sb", bufs=2) as pre_sb,
                tc.tile_pool(name="pre_ps", bufs=4, space="PSUM") as pre_ps,
            ):
                flb, flb_idx = None, -1
                for wi in range(NWIN):
                    X = wi * WIN
                    d2 = pre_ps.tile([128, WIN], f32, tag="d2")
                    cuts = sorted({X, X + WIN}
                                  | set(range((X // 512 + 1) * 512,
                                              X + WIN, 512))
                                  | set(range((X // S + 1) * S, X + WIN, S)))
                    for a, b in zip(cuts[:-1], cuts[1:]):
                        ch = a // S
                        if ch // 8 != flb_idx:
                            flb_idx = ch // 8
                            flb = pre_sb.tile([21, 1024], bf16, tag="fl")
                            nc.sync.dma_start(
                                flb[:],
                                featL_d[:, flb_idx * 1024:(flb_idx + 1) * 1024])
                        ci = ch - flb_idx * 8
                        nc.tensor.matmul(d2[:, a - X: b - X],
                                         flb[:, ci * 128:(ci + 1) * 128],
                                         featR[:, a - ch * S: b - ch * S],
                                         start=True, stop=True)
                    if _dve_window(wi):
                        # fp16 Schraudolph: u16 = sat(rne(max(d2s + B16, 0)));
                        # its piecewise-linear decode error (~3% rel) is only
                        # acceptable on a small fraction of windows.
                        nc.vector.tensor_scalar(Kbl_u16[:, X:X + WIN],
                                                d2[:], B16, 0.0,
                                                OP.add, OP.max)
                    else:
                        nc.scalar.activation(Kbl[:, X:X + WIN], d2[:],
                                             AF.Exp, scale=float(1.0 / A16))


            if dbg and dbgsel in (1, 3):
                nc.sync.dma_start(dbg_kbl.ap(), Kbl[:, 0:4 * S])

            # ---------- iterations -----------------------------------------
            # psA: persistent bilateral accumulator (first => 2048-aligned)
            psA_ctx = tc.tile_pool(name="psA", bufs=1, space="PSUM")
            psA = psA_ctx.__enter__()
            P_bl = psA.tile([LE, S], f32, tag="A")

            def chunk_matmuls(c, pieces, qsb_t):
                lhsT = qsb_t[:, c * LE:(c + 1) * LE]
                for (o, w) in pieces:
                    nc.tensor.matmul(P_bl[:, o:o + w], lhsT,
                                     Kbl[:, c * S + o:c * S + o + w],
                                     start=(c == 0), stop=(c == CH - 1))

            psum_ctx = tc.tile_pool(name="psum", bufs=1, space="PSUM")
            psum = psum_ctx.__enter__()
            # reserve the 2048B-aligned matmul-target slots first
            psum.tile([128, 512], f32, tag="mm512", bufs=2, name="mm512r")
            psum.tile([YPC, 512], f32, tag="yb", bufs=2, name="ybr")

            def yblur_t2r(t2r, t1full_t):
                """t2 = Ay^T @ t1full -> tb; DRAM-roundtrip transpose to
                [LE, YPC*W] (unaries are added later, in x-major)."""
                tb = work.tile([YPC, LE * W], f32, tag="t2b")
                t2scr = dram.tile([YPC, LE * W], f32, tag="t2scr")
                for pi, (o, w) in enumerate(T2_SLICES):
                    t2p = psum.tile([YPC, 512], f32, tag="yb", bufs=2)
                    nc.tensor.matmul(t2p[:, 0:w], Ay[:], t1full_t[:, o:o + w],
                                     start=True, stop=True)
                    if pi % 2:
                        nc.vector.tensor_copy(tb[:, o:o + w], t2p[:, 0:w])
                    else:
                        nc.scalar.copy(tb[:, o:o + w], t2p[:, 0:w])
                nc.scalar.dma_start(t2scr[:], tb[:])
                nc.sync.dma_start(
                    t2r[:].rearrange("l (y x) -> l y x", y=YPC, x=W),
                    t2scr[:].rearrange("y (l x) -> l y x", l=LE, x=W),
                )

            qag_prev = None
            for it in range(NUM_ITERATIONS):
                last = it == NUM_ITERATIONS - 1
                if it > 0:
                    qagQ, qagT = qag_prev
                    qsb = work.tile([128, CH * LE], fp16, tag="qsb")
                    t1full = work.tile([H, LE * W], f8, tag="t1full")
                    # merged reassembly: one DMA per destination tensor
                    nc.sync.dma_start(
                        qsb[:].bitcast(u8)
                        .rearrange("p (r c) -> p r c", r=NCORES),
                        qagQ[:].rearrange("r (p c) -> p r c", p=128))
                    # t1 payload is y-major so (r, y) fuses into the partition
                    # dim with a uniform stride: one DMA for all 8 cores.
                    nc.scalar.dma_start(
                        t1full[:],
                        qagT[:].rearrange("r (y c) -> (r y) c", y=YPC))
                if dbg and it == 1 and dbgsel in (1, 2):
                    nc.sync.dma_start(dbg_qsb1.ap(), qsb[:])
                    nc.sync.dma_start(dbg_t1f1.ap(), t1full[:])
                t2r = work.tile([LE, YPC * W], f32, tag="t2r")
                yblur_t2r(t2r, t1full[:])

                # ------ per-group burst (iters >= 1), then its combine /
                # softmax tail so it pipelines behind the next group's burst --
                pbs = work.tile([LE, S], f32, tag="pbs")
                qy = work.tile([W, YPC * LE], f32, tag="qy")
                qy3 = qy[:].rearrange("x (y l) -> x y l", y=YPC, l=LE)
                ssum = work.tile([W, YPC], f32, tag="ssum")
                rec12 = work.tile([W, YPC], f32, tag="rec12")
                blsc = work.tile([W, YPC * LE], f32, tag="blsc")
                blsc3 = blsc[:].rearrange("x (y l) -> x y l", y=YPC, l=LE)
                if not last:
                    qyb = work.tile([W, YPC * LE], fp16, tag="qyb")
                    t1Ts = work.tile([LE, S], f8, tag="t1Ts")
                    ql16 = work.tile([LE, S], fp16, tag="ql16")
                for gi, (y0, y1, pieces) in enumerate(GROUPS):
                    for c in range(CH):
                        chunk_matmuls(c, pieces, qsb[:])
                    c0, c1 = y0 * W, y1 * W
                    ng = y1 - y0
                    nc.scalar.copy(pbs[:, c0:c1], P_bl[:, c0:c1])
                    tp = psum.tile([W, 2 * 5 * LE], f32, tag="tp", bufs=1)
                    for k, y in enumerate(range(y0, y1)):
                        nc.tensor.transpose(
                            tp[:, (5 + k) * LE:(5 + k + 1) * LE],
                            pbs[:, y * W:(y + 1) * W], id22[:])
                        nc.tensor.transpose(tp[:, k * LE:(k + 1) * LE],
                                            t2r[:, y * W:(y + 1) * W], id22[:])
                    tp0 = tp[:, 0:ng * LE].rearrange("x (y l) -> x y l", l=LE)
                    tpB = tp[:, 5 * LE:(5 + ng) * LE].rearrange(
                        "x (y l) -> x y l", l=LE)
                    nc.vector.reciprocal(rec12[:, y0:y1][:, :, None],
                                         tpB[:, :, L:LE])
                    nc.vector.tensor_tensor(
                        blsc3[:, y0:y1], tpB,
                        rec12[:, y0:y1][:, :, None].to_broadcast([W, ng, LE]),
                        OP.mult)
                    nc.vector.tensor_tensor(tp0, tp0, uxy3[:, y0:y1], OP.add)
                    nc.vector.tensor_tensor(tp0, tp0, blsc3[:, y0:y1], OP.add)
                    nc.scalar.activation(qy[:, y0 * LE:y1 * LE],
                                         tp[:, 0:ng * LE], AF.Exp)
                    nc.vector.reduce_sum(ssum[:, y0:y1],
                                         qy3[:, y0:y1, 0:L], axis=AX.X)
                    nc.vector.reciprocal(ssum[:, y0:y1], ssum[:, y0:y1])
                    nc.vector.tensor_tensor(
                        qy3[:, y0:y1, 0:L], qy3[:, y0:y1, 0:L],
                        ssum[:, y0:y1][:, :, None].to_broadcast([W, ng, L]),
                        OP.mult)
                    if last:
                        nc.sync.dma_start(
                            qout_d.ap()[y0 * W:y1 * W, :]
                                 .rearrange("(y x) l -> x y l", x=W),
                            qy3[:, y0:y1, 0:L])
                        continue
                    nc.vector.memset(qy3[:, y0:y1, L:LE], ONESV / QSCALE)
                    nc.vector.tensor_scalar(qyb[:, y0 * LE:y1 * LE],
                                            qy[:, y0 * LE:y1 * LE],
                                            QSCALE, None, OP.mult)
                    # x-blur this group's rows into one psum bank
                    xb = psum.tile([128, 512], f32, tag="mm512", bufs=2,
                                   name="mm512r")
                    for k, y in enumerate(range(y0, y1)):
                        nc.tensor.matmul(xb[0:LE, k * W:(k + 1) * W],
                                         qyb[:, y * LE:(y + 1) * LE], Ax[:],
                                         start=True, stop=True)
                    nc.scalar.copy(t1Ts[:, c0:c1], xb[0:LE, 0:(c1 - c0)])
                    # transpose q to label-major for the chunk repack
                    qlt = psum.tile([128, 512], f32, tag="mm512", bufs=2,
                                    name="mm512r")
                    for k, y in enumerate(range(y0, y1)):
                        nc.tensor.transpose(qlt[0:LE, k * W:(k + 1) * W],
                                            qy[:, y * LE:(y + 1) * LE],
                                            id96[:])
                    nc.scalar.activation(ql16[:, c0:c1],
                                         qlt[0:LE, 0:(c1 - c0)],
                                         AF.Copy, scale=QSCALE)

                if dbg and it == 0 and dbgsel in (1, 3):
                    nc.sync.dma_start(dbg_pbl.ap(), pbs[:])
                    nc.sync.dma_start(dbg_t2r.ap(), t2r[:])
                    nc.sync.dma_start(dbg_qy.ap(), qy[:])
                if last:
                    continue

                # chunk-major repack: q128[p, c, :] = ql16[:, c*128:..]^T
                q128 = work.tile([128, 9 * LE], fp16, tag="q128")
                q128ps = psum.tile([128, 512], f32, tag="mm512", bufs=2,
                                   name="mm512r")
                for c in range(9):
                    nc.tensor.matmul(q128ps[:, c * LE:(c + 1) * LE],
                                     ql16[:, c * 128:(c + 1) * 128], i22h[:],
                                     start=True, stop=True)
                nc.scalar.copy(q128[:], q128ps[:, 0:9 * LE])

                # payload staging (t1 part re-laid y-major) + AllGather
                plQ = dram.tile([1, PAYQ], u8, tag="plQ")
                plT = dram.tile([1, PAYT], f8, tag="plT")
                nc.sync.dma_start(
                    plQ[0:1, :].rearrange("a (p c) -> (a p) c", p=128),
                    q128[:].bitcast(u8))
                nc.scalar.dma_start(
                    plT[0:1, :].rearrange("a (y l x) -> (a l) y x",
                                          y=YPC, l=LE),
                    t1Ts[:].rearrange("l (y x) -> l y x", y=YPC))
                qagQ = dram.tile([NCORES, PAYQ], u8, tag="qagQ")
                qagT = dram.tile([NCORES, PAYT], f8, tag="qagT")
                if sim1:
                    nc.sync.dma_start(
                        qagQ[:], plQ[0:1, :].to_broadcast([NCORES, PAYQ]))
                    nc.scalar.dma_start(
                        qagT[:], plT[0:1, :].to_broadcast([NCORES, PAYT]))
                else:
                    nc.gpsimd.collective_compute(
                        "AllGather", OP.bypass,
                        replica_groups=[list(range(NCORES))],
                        ins=[plQ.opt()], outs=[qagQ.opt()])
                    nc.gpsimd.collective_compute(
                        "AllGather", OP.bypass,
                        replica_groups=[list(range(NCORES))],
                        ins=[plT.opt()], outs=[qagT.opt()])
                qag_prev = (qagQ, qagT)
            psum_ctx.__exit__(None, None, None)
            psA_ctx.__exit__(None, None, None)

    nc.compile()
    _CACHE[key] = nc
    return nc


def _host_prepare(unaries, rgb):
    u = np.asarray(unaries, np.float32).reshape(N, L)
    c = np.asarray(rgb, np.float32).reshape(N, 3)
    bfd = ml_dtypes.bfloat16
    f8d = ml_dtypes.float8_e4m3
    f16 = np.float16

    ys, xs = np.meshgrid(np.arange(H, dtype=np.float64),
                         np.arange(W, dtype=np.float64), indexing="ij")
    pos = np.stack([ys.ravel(), xs.ravel()], -1)            # [N, 2]
    g = np.concatenate([c.astype(np.float64) / BETA, pos / ALPHA], 1)
    g = g - g.mean(0, keepdims=True)
    sq = (g * g).sum(1)
    ones = np.ones(N, np.float64)
    L7 = np.concatenate([g.T, ones[None], (-0.5 * sq)[None]], 0)         # [7,N]
    R7 = np.concatenate([g.T, (-0.5 * sq)[None], ones[None]], 0) * A16   # [7,N]
    Lhi = L7.astype(bfd)
    Llo = (L7 - Lhi.astype(np.float64)).astype(bfd)
    Rhi = R7.astype(bfd)
    Rlo = (R7 - Rhi.astype(np.float64)).astype(bfd)
    # dot = Lhi.Rhi + Lhi.Rlo + Llo.Rhi  (Llo.Rlo dropped, ~1e-3)
    featL = np.ascontiguousarray(np.concatenate([Lhi, Lhi, Llo], 0))  # [21,N]
    featR = np.ascontiguousarray(np.concatenate([Rhi, Rlo, Rhi], 0))  # [21,N]

    d = np.arange(W, dtype=np.float64)
    A = np.exp(-(d[:, None] - d[None, :]) ** 2 / (2.0 * GAMMA * GAMMA))
    nvec = A.sum(0)
    Ax = np.ascontiguousarray((A / nvec[None, :]).astype(bfd))

    um = u.max(1, keepdims=True)
    e = np.exp(u - um)
    q0 = e / e.sum(1, keepdims=True)
    q0s = np.concatenate([QSCALE * q0, np.full((N, 1), ONESV, np.float64)],
                         1)                                   # [N, 22] scaled
    qsb0 = np.ascontiguousarray(
        q0s.reshape(CH, 128, LE).transpose(1, 0, 2)
        .reshape(128, CH * LE)).astype(f16)

    q3 = q0s.reshape(H, W, LE)
    t1 = np.einsum("Xx,yXl->ylx", A / nvec[None, :], q3)      # [96, 22, 96]
    t1f0 = np.ascontiguousarray(t1.reshape(H, LE * W).astype(f8d))

    id22 = np.eye(LE, dtype=np.float32)
    id96 = np.eye(W, dtype=np.float32)
    i22h = np.eye(LE, dtype=np.float32).astype(f16)

    in_maps = []
    for core in range(NCORES):
        rows = slice(core * S, (core + 1) * S)
        ue = np.full((S, LE), -50.0, np.float32)
        ue[:, 0:L] = u[rows]
        # x-major unaries: uxy[x, y*LE + l] = ue[y*W + x, l]
        uxy_c = np.ascontiguousarray(
            ue.reshape(YPC, W, LE).transpose(1, 0, 2).reshape(W, YPC * LE))
        yc = slice(core * YPC, (core + 1) * YPC)
        Ay_c = np.ascontiguousarray(
            (A[:, yc] * (W_SPATIAL / QSCALE / nvec[yc])[None, :]).astype(bfd))
        in_maps.append({
            "featL": featL,
            "featR": np.ascontiguousarray(featR[:, rows]),
            "uxy": uxy_c,
            "Ax": Ax,
            "Ay": Ay_c,
            "qsb0": qsb0,
            "t1f0": t1f0,
            "id22": id22,
            "id96": id96,
            "i22h": i22h,
        })
    return in_maps


def _get_runner():
    """Compile once; return (fn, in_names, out_names) where fn maps
    concatenated global numpy inputs -> list of per-core output dicts."""
    if "runner" in _CACHE:
        return _CACHE["runner"]
    import jax
    from jax.sharding import Mesh, PartitionSpec
    from jax.experimental.shard_map import shard_map
    import concourse.mybir as mybir
    from concourse import bass2jax

    nc = _build_bass()
    bass2jax.install_neuronx_cc_hook()

    partition_name = (nc.partition_id_tensor.name
                      if nc.partition_id_tensor else None)
    in_names, out_names, out_avals, zero_outs = [], [], [], []
    for alloc in nc.m.functions[0].allocations:
        if not isinstance(alloc, mybir.MemoryLocationSet):
            continue
        name = alloc.memorylocations[0].name
        if alloc.kind == "ExternalInput":
            if name != partition_name:
                in_names.append(name)
        elif alloc.kind == "ExternalOutput":
            shape = tuple(alloc.tensor_shape)
            dtype = mybir.dt.np(alloc.dtype)
            out_names.append(name)
            out_avals.append(jax.core.ShapedArray(shape, dtype))
            zero_outs.append(np.zeros(shape, dtype))
    n_params = len(in_names)
    all_in_names = list(in_names) + list(out_names)
    if partition_name is not None:
        all_in_names.append(partition_name)

    def _body(*args):
        operands = list(args)
        if partition_name is not None:
            operands.append(bass2jax.partition_id_tensor())
        outs = bass2jax._bass_exec_p.bind(
            *operands,
            out_avals=tuple(out_avals),
            in_names=tuple(all_in_names),
            out_names=tuple(out_names),
            lowering_input_output_aliases=(),
            sim_require_finite=False,
            sim_require_nnan=False,
            nc=nc,
        )
        return tuple(outs)

    devices = jax.devices()[:NCORES]
    mesh = Mesh(np.asarray(devices), ("core",))
    n_outs = len(out_names)
    in_specs = (PartitionSpec("core"),) * (n_params + n_outs)
    out_specs = (PartitionSpec("core"),) * n_outs
    donate = tuple(range(n_params, n_params + n_outs))
    fn = jax.jit(
        shard_map(_body, mesh=mesh, in_specs=in_specs, out_specs=out_specs,
                  check_rep=False),
        donate_argnums=donate, keep_unused=True)
    _CACHE["runner"] = (fn, in_names, out_names, out_avals, zero_outs)
    return _CACHE["runner"]


def _concat_inputs(in_maps, in_names):
    return [np.concatenate([np.asarray(in_maps[c][nm]) for c in range(NCORES)],
                           axis=0) for nm in in_names]


def _run(in_maps):
    fn, in_names, out_names, out_avals, zero_outs = _get_runner()
    concat_in = _concat_inputs(in_maps, in_names)
    concat_zeros = [np.zeros((NCORES * z.shape[0], *z.shape[1:]), z.dtype)
                    for z in zero_outs]
    out_arrs = fn(*concat_in, *concat_zeros)
    return out_arrs, out_names, out_avals


def kernel(unaries, rgb):
    in_maps = _host_prepare(unaries, rgb)
    out_arrs, out_names, out_avals = _run(in_maps)
    qi = out_names.index("qout")
    q = np.asarray(out_arrs[qi]).reshape(NCORES, S, L).reshape(N, L)
    return np.ascontiguousarray(q[None].astype(np.float32))


def time_kernel(unaries, rgb, iters=20):
    """Steady-state per-call wall time of the compiled 8-core executable,
    with inputs pre-staged on device."""
    import time as _time
    import jax
    in_maps = _host_prepare(unaries, rgb)
    fn, in_names, out_names, out_avals, zero_outs = _get_runner()
    concat_in = _concat_inputs(in_maps, in_names)

    def once():
        concat_zeros = [np.zeros((NCORES * z.shape[0], *z.shape[1:]), z.dtype)
                        for z in zero_outs]
        outs = fn(*concat_in, *concat_zeros)
        jax.block_until_ready(outs)
        return outs

    once()  # warm
    times = []
    for _ in range(iters):
        t0 = _time.perf_counter()
        once()
        times.append(_time.perf_counter() - t0)
    return min(times), sorted(times)[len(times) // 2]


# revision 33
# speedup vs baseline: 1.0510x; 1.0510x over previous
"""Trainium2 Bass kernel: dense-CRF mean-field layer (96x96 image, 21 labels).

Strategy (8 NeuronCores, row-sharded):
  * K_bl [N, S-slice] built once on-device in fp16 (fp8 fails the accuracy
    budget: the mean-field dynamics amplify ~3% kernel noise into ~3e-2
    worst-pixel errors) and kept SBUF-resident per core.
  * The K_bl build (d2 matmul + Exp on the scalar engine) is fused with
    iteration-0's bilateral accumulation: chunk matmuls fire as their K_bl
    columns come out of the exp, hiding all of iteration 0's PE work inside
    the ACT-bound exp window.  Optionally a small fraction of exp windows
    run on the vector engine as a 16-bit Schraudolph (u16 bitcast fp16).
  * q carries a 22nd norm channel = 0.125 and labels pre-scaled by 1.25, so
    P_l / P_norm = 10 * message (W_BILATERAL folded); the spatial blur
    matrices absorb the 1/1.25.
  * Spatial kernel is separable: x-blur per core pre-gather, y-blur
    post-gather (W_SPATIAL/norm folded into host blur matrices); the
    x-blurred t1 rides the AllGather in fp8 (accuracy-checked).
  * Per-iteration tail is group-pipelined (3 y-groups: burst -> transpose ->
    softmax -> x-blur while the next group's burst runs), q is repacked to
    chunk-major via PE transposes + a repack matmul (no partition-shift DMA
    dance), and the gathered payloads are reassembled with one merged DMA
    per destination tensor.
"""
import sys
sys.path.insert(0, "/opt/trn_rl_repo")
import os
import numpy as np
import ml_dtypes

H = W = 96
N = H * W                  # 9216
L = 21
LE = L + 1                 # 22 channels (21 labels + norm channel)
ALPHA, BETA, GAMMA = 80.0, 13.0, 3.0
W_SPATIAL, W_BILATERAL = 3.0, 10.0
NUM_ITERATIONS = 5
NCORES = 8
S = N // NCORES            # 1152 rows per core
YPC = H // NCORES          # 12 image rows per core
CH = N // 128              # 72 chunks of 128 rows (global j)
KCOLS = CH * S             # 82944 K_bl columns (fp16: 2B each)
WIN = 1024                 # d2/exp window (81 windows exactly)
NWIN = KCOLS // WIN
PAYQ = 128 * 9 * LE * 2    # 50688 payload bytes (q part, fp16)
PAYT = S * LE              # 25344 payload bytes (t1 part, fp8)
A16 = 1024.0 / float(np.log(2.0))   # fp16 Schraudolph scale (10 mantissa bits)
B16 = float(15 * 1024)              # fp16 exponent bias 15 << 10
ONESV = 0.125              # q norm-channel value
QSCALE = 1.25              # label-channel pre-scale; 1.25/0.125 = W_BILATERAL
DVE_SHARE = float(os.environ.get("CRF_DVE_SHARE", "0.15"))

# Bilateral output groups (by y-rows of this core's slice) for tail pipelining
GROUPS = (
    (0, 5, ((0, 480),)),
    (5, 10, ((480, 32), (512, 448))),
    (10, 12, ((960, 64), (1024, 128))),
)
T2_SLICES = [(0, 512), (512, 512), (1024, 512), (1536, 512), (2048, 64)]

LAST_EXEC_NS = None
_CACHE = {}


def _dve_window(w):
    """Deterministic ACT/DVE interleave for exp windows."""
    return int((w + 1) * DVE_SHARE) != int(w * DVE_SHARE)


def _build_bass(sim1=False):
    """Build the kernel. sim1=True builds a single-core variant where the
    AllGather is replaced by a broadcast DRAM copy (for TimelineSim)."""
    key = "nc_sim1" if sim1 else "nc"
    if key in _CACHE:
        return _CACHE[key]
    import concourse.bass as bass  # noqa: F401
    from concourse import bacc
    import concourse.mybir as mybir
    import concourse.tile as tile

    f32 = mybir.dt.float32
    bf16 = mybir.dt.bfloat16
    fp16 = mybir.dt.float16
    f8 = mybir.dt.float8e4
    u16 = mybir.dt.uint16
    u8 = mybir.dt.uint8
    AF = mybir.ActivationFunctionType
    OP = mybir.AluOpType
    AX = mybir.AxisListType

    nc = bacc.Bacc("TRN2", target_bir_lowering=False, debug=False,
                   num_devices=1 if sim1 else NCORES)

    featL_d = nc.dram_tensor("featL", [21, N], bf16, kind="ExternalInput")
    featR_d = nc.dram_tensor("featR", [21, S], bf16, kind="ExternalInput")
    uxy_d = nc.dram_tensor("uxy", [W, YPC * LE], f32, kind="ExternalInput")
    Ax_d = nc.dram_tensor("Ax", [W, W], bf16, kind="ExternalInput")
    Ay_d = nc.dram_tensor("Ay", [H, YPC], bf16, kind="ExternalInput")
    qsb0_d = nc.dram_tensor("qsb0", [128, CH * LE], fp16, kind="ExternalInput")
    t1f0_d = nc.dram_tensor("t1f0", [H, LE * W], f8, kind="ExternalInput")
    id22_d = nc.dram_tensor("id22", [LE, LE], f32, kind="ExternalInput")
    id96_d = nc.dram_tensor("id96", [W, W], f32, kind="ExternalInput")
    i22h_d = nc.dram_tensor("i22h", [LE, LE], fp16, kind="ExternalInput")
    qout_d = nc.dram_tensor("qout", [S, L], f32, kind="ExternalOutput")
    dbgsel = int(os.environ.get("CRF_DEBUG", "0"))
    dbg = dbgsel > 0
    if dbg:
        dbg_kbl = nc.dram_tensor("dbg_kbl", [128, 4 * S], fp16,
                                 kind="ExternalOutput")
        dbg_pbl = nc.dram_tensor("dbg_pbl", [LE, S], f32, kind="ExternalOutput")
        dbg_t2r = nc.dram_tensor("dbg_t2r", [LE, YPC * W], f32,
                                 kind="ExternalOutput")
        dbg_qy = nc.dram_tensor("dbg_qy", [W, YPC * LE], f32,
                                kind="ExternalOutput")
        dbg_qsb1 = nc.dram_tensor("dbg_qsb1", [128, CH * LE], fp16,
                                  kind="ExternalOutput")
        dbg_t1f1 = nc.dram_tensor("dbg_t1f1", [H, LE * W], f8,
                                  kind="ExternalOutput")

    with tile.TileContext(nc) as tc:
        with (
            tc.tile_pool(name="const", bufs=1) as constp,
            tc.tile_pool(name="kbl", bufs=1) as kblp,
            tc.tile_pool(name="work", bufs=1) as work,
            tc.tile_pool(name="dram", bufs=2, space="DRAM") as dram,
        ):
            Ax = constp.tile([W, W], bf16)
            nc.sync.dma_start(Ax[:], Ax_d[:])
            Ay = constp.tile([H, YPC], bf16)
            nc.sync.dma_start(Ay[:], Ay_d[:])
            uxy = constp.tile([W, YPC * LE], f32)
            nc.sync.dma_start(uxy[:], uxy_d[:])
            uxy3 = uxy[:].rearrange("x (y l) -> x y l", y=YPC, l=LE)
            id22 = constp.tile([LE, LE], f32)
            nc.sync.dma_start(id22[:], id22_d[:])
            id96 = constp.tile([W, W], f32)
            nc.sync.dma_start(id96[:], id96_d[:])
            i22h = constp.tile([LE, LE], fp16)
            nc.sync.dma_start(i22h[:], i22h_d[:])
            featR = constp.tile([21, S], bf16)
            nc.sync.dma_start(featR[:], featR_d[:])
            Kbl = kblp.tile([128, KCOLS], fp16)
            Kbl_u16 = Kbl[:].bitcast(u16)

            qsb = work.tile([128, CH * LE], fp16, tag="qsb")
            t1full = work.tile([H, LE * W], f8, tag="t1full")
            nc.sync.dma_start(qsb[:], qsb0_d[:])
            nc.scalar.dma_start(t1full[:], t1f0_d[:])

            # ------- precompute K_bl (deep-pipelined over all 8 banks) ------
            with (
                tc.tile_pool(name="pre_sb", bufs=2) as pre_sb,
                tc.tile_pool(name="pre_ps", bufs=4, space="PSUM") as pre_ps,
            ):
                flb, flb_idx = None, -1
                for wi in range(NWIN):
                    X = wi * WIN
                    d2 = pre_ps.tile([128, WIN], f32, tag="d2")
                    cuts = sorted({X, X + WIN}
                                  | set(range((X // 512 + 1) * 512,
                                              X + WIN, 512))
                                  | set(range((X // S + 1) * S, X + WIN, S)))
                    for a, b in zip(cuts[:-1], cuts[1:]):
                        ch = a // S
                        if ch // 8 != flb_idx:
                            flb_idx = ch // 8
                            flb = pre_sb.tile([21, 1024], bf16, tag="fl")
                            nc.sync.dma_start(
                                flb[:],
                                featL_d[:, flb_idx * 1024:(flb_idx + 1) * 1024])
                        ci = ch - flb_idx * 8
                        nc.tensor.matmul(d2[:, a - X: b - X],
                                         flb[:, ci * 128:(ci + 1) * 128],
                                         featR[:, a - ch * S: b - ch * S],
                                         start=True, stop=True)
                    if _dve_window(wi):
                        # fp16 Schraudolph: u16 = sat(rne(max(d2s + B16, 0)));
                        # its piecewise-linear decode error (~3% rel) is only
                        # acceptable on a small fraction of windows.
                        nc.vector.tensor_scalar(Kbl_u16[:, X:X + WIN],
                                                d2[:], B16, 0.0,
                                                OP.add, OP.max)
                    else:
                        nc.scalar.activation(Kbl[:, X:X + WIN], d2[:],
                                             AF.Exp, scale=float(1.0 / A16))


            if dbg and dbgsel in (1, 3):
                nc.sync.dma_start(dbg_kbl.ap(), Kbl[:, 0:4 * S])

            # ---------- iterations -----------------------------------------
            # psA: persistent bilateral accumulator (first => 2048-aligned)
            psA_ctx = tc.tile_pool(name="psA", bufs=1, space="PSUM")
            psA = psA_ctx.__enter__()
            P_bl = psA.tile([LE, S], f32, tag="A")

            def chunk_matmuls(c, pieces, qsb_t):
                lhsT = qsb_t[:, c * LE:(c + 1) * LE]
                for (o, w) in pieces:
                    nc.tensor.matmul(P_bl[:, o:o + w], lhsT,
                                     Kbl[:, c * S + o:c * S + o + w],
                                     start=(c == 0), stop=(c == CH - 1))

            psum_ctx = tc.tile_pool(name="psum", bufs=1, space="PSUM")
            psum = psum_ctx.__enter__()
            # reserve the 2048B-aligned matmul-target slots first
            psum.tile([128, 512], f32, tag="mm512", bufs=2, name="mm512r")
            psum.tile([YPC, 512], f32, tag="yb", bufs=2, name="ybr")

            def yblur_t2r(t2r, t1full_t):
                """t2 = Ay^T @ t1full -> tb; DRAM-roundtrip transpose to
                [LE, YPC*W] (unaries are added later, in x-major)."""
                tb = work.tile([YPC, LE * W], f32, tag="t2b")
                t2scr = dram.tile([YPC, LE * W], f32, tag="t2scr")
                for pi, (o, w) in enumerate(T2_SLICES):
                    t2p = psum.tile([YPC, 512], f32, tag="yb", bufs=2)
                    nc.tensor.matmul(t2p[:, 0:w], Ay[:], t1full_t[:, o:o + w],
                                     start=True, stop=True)
                    if pi % 2:
                        nc.vector.tensor_copy(tb[:, o:o + w], t2p[:, 0:w])
                    else:
                        nc.scalar.copy(tb[:, o:o + w], t2p[:, 0:w])
                nc.scalar.dma_start(t2scr[:], tb[:])
                nc.sync.dma_start(
                    t2r[:].rearrange("l (y x) -> l y x", y=YPC, x=W),
                    t2scr[:].rearrange("y (l x) -> l y x", l=LE, x=W),
                )

            qag_prev = None
            for it in range(NUM_ITERATIONS):
                last = it == NUM_ITERATIONS - 1
                if it > 0:
                    qagQ, qagT = qag_prev
                    qsb = work.tile([128, CH * LE], fp16, tag="qsb")
                    t1full = work.tile([H, LE * W], f8, tag="t1full")
                    # merged reassembly: one DMA per destination tensor
                    nc.sync.dma_start(
                        qsb[:].bitcast(u8)
                        .rearrange("p (r c) -> p r c", r=NCORES),
                        qagQ[:].rearrange("r (p c) -> p r c", p=128))
                    # t1 payload is y-major so (r, y) fuses into the partition
                    # dim with a uniform stride: one DMA for all 8 cores.
                    nc.scalar.dma_start(
                        t1full[:],
                        qagT[:].rearrange("r (y c) -> (r y) c", y=YPC))
                if dbg and it == 1 and dbgsel in (1, 2):
                    nc.sync.dma_start(dbg_qsb1.ap(), qsb[:])
                    nc.sync.dma_start(dbg_t1f1.ap(), t1full[:])
                t2r = work.tile([LE, YPC * W], f32, tag="t2r")
                yblur_t2r(t2r, t1full[:])

                # ------ per-group burst (iters >= 1), then its combine /
                # softmax tail so it pipelines behind the next group's burst --
                pbs = work.tile([LE, S], f32, tag="pbs")
                qy = work.tile([W, YPC * LE], f32, tag="qy")
                qy3 = qy[:].rearrange("x (y l) -> x y l", y=YPC, l=LE)
                ssum = work.tile([W, YPC], f32, tag="ssum")
                rec12 = work.tile([W, YPC], f32, tag="rec12")
                blsc = work.tile([W, YPC * LE], f32, tag="blsc")
                blsc3 = blsc[:].rearrange("x (y l) -> x y l", y=YPC, l=LE)
                if not last:
                    qyb = work.tile([W, YPC * LE], fp16, tag="qyb")
                    t1Ts = work.tile([LE, S], f8, tag="t1Ts")
                    ql16 = work.tile([LE, S], fp16, tag="ql16")
                for gi, (y0, y1, pieces) in enumerate(GROUPS):
                    for c in range(CH):
                        chunk_matmuls(c, pieces, qsb[:])
                    c0, c1 = y0 * W, y1 * W
                    ng = y1 - y0
                    nc.scalar.copy(pbs[:, c0:c1], P_bl[:, c0:c1])
                    tp = psum.tile([W, 2 * 5 * LE], f32, tag="tp", bufs=1)
                    for k, y in enumerate(range(y0, y1)):
                        nc.tensor.transpose(
                            tp[:, (5 + k) * LE:(5 + k + 1) * LE],
                            pbs[:, y * W:(y + 1) * W], id22[:])
                        nc.tensor.transpose(tp[:, k * LE:(k + 1) * LE],
                                            t2r[:, y * W:(y + 1) * W], id22[:])
                    tp0 = tp[:, 0:ng * LE].rearrange("x (y l) -> x y l", l=LE)
                    tpB = tp[:, 5 * LE:(5 + ng) * LE].rearrange(
                        "x (y l) -> x y l", l=LE)
                    nc.vector.reciprocal(rec12[:, y0:y1][:, :, None],
                                         tpB[:, :, L:LE])
                    nc.vector.tensor_tensor(
                        blsc3[:, y0:y1], tpB,
                        rec12[:, y0:y1][:, :, None].to_broadcast([W, ng, LE]),
                        OP.mult)
                    nc.vector.tensor_tensor(tp0, tp0, uxy3[:, y0:y1], OP.add)
                    nc.vector.tensor_tensor(tp0, tp0, blsc3[:, y0:y1], OP.add)
                    nc.scalar.activation(qy[:, y0 * LE:y1 * LE],
                                         tp[:, 0:ng * LE], AF.Exp)
                    nc.vector.reduce_sum(ssum[:, y0:y1],
                                         qy3[:, y0:y1, 0:L], axis=AX.X)
                    nc.vector.reciprocal(ssum[:, y0:y1], ssum[:, y0:y1])
                    nc.vector.tensor_tensor(
                        qy3[:, y0:y1, 0:L], qy3[:, y0:y1, 0:L],
                        ssum[:, y0:y1][:, :, None].to_broadcast([W, ng, L]),
                        OP.mult)
                    if last:
                        nc.sync.dma_start(
                            qout_d.ap()[y0 * W:y1 * W, :]
                                 .rearrange("(y x) l -> x y l", x=W),
                            qy3[:, y0:y1, 0:L])
                        continue
                    nc.vector.memset(qy3[:, y0:y1, L:LE], ONESV / QSCALE)
                    nc.vector.tensor_scalar(qyb[:, y0 * LE:y1 * LE],
                                            qy[:, y0 * LE:y1 * LE],
                                            QSCALE, None, OP.mult)
                    # x-blur this group's rows into one psum bank
                    xb = psum.tile([128, 512], f32, tag="mm512", bufs=2,
                                   name="mm512r")
                    for k, y in enumerate(range(y0, y1)):
                        nc.tensor.matmul(xb[0:LE, k * W:(k + 1) * W],
                                         qyb[:, y * LE:(y + 1) * LE], Ax[:],
                                         start=True, stop=True)
                    nc.scalar.copy(t1Ts[:, c0:c1], xb[0:LE, 0:(c1 - c0)])
                    # transpose q to label-major for the chunk repack
                    qlt = psum.tile([128, 512], f32, tag="mm512", bufs=2,
                                    name="mm512r")
                    for k, y in enumerate(range(y0, y1)):
                        nc.tensor.transpose(qlt[0:LE, k * W:(k + 1) * W],
                                            qy[:, y * LE:(y + 1) * LE],
                                            id96[:])
                    nc.scalar.activation(ql16[:, c0:c1],
                                         qlt[0:LE, 0:(c1 - c0)],
                                         AF.Copy, scale=QSCALE)

                if dbg and it == 0 and dbgsel in (1, 3):
                    nc.sync.dma_start(dbg_pbl.ap(), pbs[:])
                    nc.sync.dma_start(dbg_t2r.ap(), t2r[:])
                    nc.sync.dma_start(dbg_qy.ap(), qy[:])
                if last:
                    continue

                # chunk-major repack: q128[p, c, :] = ql16[:, c*128:..]^T
                q128 = work.tile([128, 9 * LE], fp16, tag="q128")
                q128ps = psum.tile([128, 512], f32, tag="mm512", bufs=2,
                                   name="mm512r")
                for c in range(9):
                    nc.tensor.matmul(q128ps[:, c * LE:(c + 1) * LE],
                                     ql16[:, c * 128:(c + 1) * 128], i22h[:],
                                     start=True, stop=True)
                nc.scalar.copy(q128[:], q128ps[:, 0:9 * LE])

                # payload staging (t1 part re-laid y-major) + AllGather
                plQ = dram.tile([1, PAYQ], u8, tag="plQ")
                plT = dram.tile([1, PAYT], f8, tag="plT")
                nc.sync.dma_start(
                    plQ[0:1, :].rearrange("a (p c) -> (a p) c", p=128),
                    q128[:].bitcast(u8))
                nc.scalar.dma_start(
                    plT[0:1, :].rearrange("a (y l x) -> (a l) y x",
                                          y=YPC, l=LE),
                    t1Ts[:].rearrange("l (y x) -> l y x", y=YPC))
                qagQ = dram.tile([NCORES, PAYQ], u8, tag="qagQ")
                qagT = dram.tile([NCORES, PAYT], f8, tag="qagT")
                if sim1:
                    nc.sync.dma_start(
                        qagQ[:], plQ[0:1, :].to_broadcast([NCORES, PAYQ]))
                    nc.scalar.dma_start(
                        qagT[:], plT[0:1, :].to_broadcast([NCORES, PAYT]))
                else:
                    nc.gpsimd.collective_compute(
                        "AllGather", OP.bypass,
                        replica_groups=[list(range(NCORES))],
                        ins=[plQ.opt()], outs=[qagQ.opt()])
                    nc.gpsimd.collective_compute(
                        "AllGather", OP.bypass,
                        replica_groups=[list(range(NCORES))],
                        ins=[plT.opt()], outs=[qagT.opt()])
                qag_prev = (qagQ, qagT)
            psum_ctx.__exit__(None, None, None)
            psA_ctx.__exit__(None, None, None)

    nc.compile()
    _CACHE[key] = nc
    return nc


def _host_prepare(unaries, rgb):
    u = np.asarray(unaries, np.float32).reshape(N, L)
    c = np.asarray(rgb, np.float32).reshape(N, 3)
    bfd = ml_dtypes.bfloat16
    f8d = ml_dtypes.float8_e4m3
    f16 = np.float16

    ys, xs = np.meshgrid(np.arange(H, dtype=np.float64),
                         np.arange(W, dtype=np.float64), indexing="ij")
    pos = np.stack([ys.ravel(), xs.ravel()], -1)            # [N, 2]
    g = np.concatenate([c.astype(np.float64) / BETA, pos / ALPHA], 1)
    g = g - g.mean(0, keepdims=True)
    sq = (g * g).sum(1)
    ones = np.ones(N, np.float64)
    L7 = np.concatenate([g.T, ones[None], (-0.5 * sq)[None]], 0)         # [7,N]
    R7 = np.concatenate([g.T, (-0.5 * sq)[None], ones[None]], 0) * A16   # [7,N]
    Lhi = L7.astype(bfd)
    Llo = (L7 - Lhi.astype(np.float64)).astype(bfd)
    Rhi = R7.astype(bfd)
    Rlo = (R7 - Rhi.astype(np.float64)).astype(bfd)
    # dot = Lhi.Rhi + Lhi.Rlo + Llo.Rhi  (Llo.Rlo dropped, ~1e-3)
    featL = np.ascontiguousarray(np.concatenate([Lhi, Lhi, Llo], 0))  # [21,N]
    featR = np.ascontiguousarray(np.concatenate([Rhi, Rlo, Rhi], 0))  # [21,N]

    d = np.arange(W, dtype=np.float64)
    A = np.exp(-(d[:, None] - d[None, :]) ** 2 / (2.0 * GAMMA * GAMMA))
    nvec = A.sum(0)
    Ax = np.ascontiguousarray((A / nvec[None, :]).astype(bfd))

    um = u.max(1, keepdims=True)
    e = np.exp(u - um)
    q0 = e / e.sum(1, keepdims=True)
    q0s = np.concatenate([QSCALE * q0, np.full((N, 1), ONESV, np.float64)],
                         1)                                   # [N, 22] scaled
    qsb0 = np.ascontiguousarray(
        q0s.reshape(CH, 128, LE).transpose(1, 0, 2)
        .reshape(128, CH * LE)).astype(f16)

    q3 = q0s.reshape(H, W, LE)
    t1 = np.einsum("Xx,yXl->ylx", A / nvec[None, :], q3)      # [96, 22, 96]
    t1f0 = np.ascontiguousarray(t1.reshape(H, LE * W).astype(f8d))

    id22 = np.eye(LE, dtype=np.float32)
    id96 = np.eye(W, dtype=np.float32)
    i22h = np.eye(LE, dtype=np.float32).astype(f16)

    in_maps = []
    for core in range(NCORES):
        rows = slice(core * S, (core + 1) * S)
        ue = np.full((S, LE), -50.0, np.float32)
        ue[:, 0:L] = u[rows]
        # x-major unaries: uxy[x, y*LE + l] = ue[y*W + x, l]
        uxy_c = np.ascontiguousarray(
            ue.reshape(YPC, W, LE).transpose(1, 0, 2).reshape(W, YPC * LE))
        yc = slice(core * YPC, (core + 1) * YPC)
        Ay_c = np.ascontiguousarray(
            (A[:, yc] * (W_SPATIAL / QSCALE / nvec[yc])[None, :]).astype(bfd))
        in_maps.append({
            "featL": featL,
            "featR": np.ascontiguousarray(featR[:, rows]),
            "uxy": uxy_c,
            "Ax": Ax,
            "Ay": Ay_c,
            "qsb0": qsb0,
            "t1f0": t1f0,
            "id22": id22,
            "id96": id96,
            "i22h": i22h,
        })
    return in_maps


def _get_runner():
    """Compile once; return (fn, in_names, out_names) where fn maps
    concatenated global numpy inputs -> list of per-core output dicts."""
    if "runner" in _CACHE:
        return _CACHE["runner"]
    import jax
    from jax.sharding import Mesh, PartitionSpec
    from jax.experimental.shard_map import shard_map
    import concourse.mybir as mybir
    from concourse import bass2jax

    nc = _build_bass()
    bass2jax.install_neuronx_cc_hook()

    partition_name = (nc.partition_id_tensor.name
                      if nc.partition_id_tensor else None)
    in_names, out_names, out_avals, zero_outs = [], [], [], []
    for alloc in nc.m.functions[0].allocations:
        if not isinstance(alloc, mybir.MemoryLocationSet):
            continue
        name = alloc.memorylocations[0].name
        if alloc.kind == "ExternalInput":
            if name != partition_name:
                in_names.append(name)
        elif alloc.kind == "ExternalOutput":
            shape = tuple(alloc.tensor_shape)
            dtype = mybir.dt.np(alloc.dtype)
            out_names.append(name)
            out_avals.append(jax.core.ShapedArray(shape, dtype))
            zero_outs.append(np.zeros(shape, dtype))
    n_params = len(in_names)
    all_in_names = list(in_names) + list(out_names)
    if partition_name is not None:
        all_in_names.append(partition_name)

    def _body(*args):
        operands = list(args)
        if partition_name is not None:
            operands.append(bass2jax.partition_id_tensor())
        outs = bass2jax._bass_exec_p.bind(
            *operands,
            out_avals=tuple(out_avals),
            in_names=tuple(all_in_names),
            out_names=tuple(out_names),
            lowering_input_output_aliases=(),
            sim_require_finite=False,
            sim_require_nnan=False,
            nc=nc,
        )
        return tuple(outs)

    devices = jax.devices()[:NCORES]
    mesh = Mesh(np.asarray(devices), ("core",))
    n_outs = len(out_names)
    in_specs = (PartitionSpec("core"),) * (n_params + n_outs)
    out_specs = (PartitionSpec("core"),) * n_outs
    donate = tuple(range(n_params, n_params + n_outs))
    fn = jax.jit(
        shard_map(_body, mesh=mesh, in_specs=in_specs, out_specs=out_specs,
                  check_rep=False),
        donate_argnums=donate, keep_unused=True)
    _CACHE["runner"] = (fn, in_names, out_names, out_avals, zero_outs)
    return _CACHE["runner"]


def _concat_inputs(in_maps, in_names):
    return [np.concatenate([np.asarray(in_maps[c][nm]) for c in range(NCORES)],
                           axis=0) for nm in in_names]


def _run(in_maps):
    fn, in_names, out_names, out_avals, zero_outs = _get_runner()
    concat_in = _concat_inputs(in_maps, in_names)
    concat_zeros = [np.zeros((NCORES * z.shape[0], *z.shape[1:]), z.dtype)
                    for z in zero_outs]
    out_arrs = fn(*concat_in, *concat_zeros)
    return out_arrs, out_names, out_avals


def kernel(unaries, rgb):
    in_maps = _host_prepare(unaries, rgb)
    out_arrs, out_names, out_avals = _run(in_maps)
    qi = out_names.index("qout")
    q = np.asarray(out_arrs[qi]).reshape(NCORES, S, L).reshape(N, L)
    return np.ascontiguousarray(q[None].astype(np.float32))


def time_kernel(unaries, rgb, iters=20):
    """Steady-state per-call wall time of the compiled 8-core executable,
    with inputs pre-staged on device."""
    import time as _time
    import jax
    in_maps = _host_prepare(unaries, rgb)
    fn, in_names, out_names, out_avals, zero_outs = _get_runner()
    concat_in = _concat_inputs(in_maps, in_names)

    def once():
        concat_zeros = [np.zeros((NCORES * z.shape[0], *z.shape[1:]), z.dtype)
                        for z in zero_outs]
        outs = fn(*concat_in, *concat_zeros)
        jax.block_until_ready(outs)
        return outs

    once()  # warm
    times = []
    for _ in range(iters):
        t0 = _time.perf_counter()
        once()
        times.append(_time.perf_counter() - t0)
    return min(times), sorted(times)[len(times) // 2]


# revision 34
# speedup vs baseline: 1.0735x; 1.0214x over previous
"""Trainium2 Bass kernel: dense-CRF mean-field layer (96x96 image, 21 labels).

Strategy (8 NeuronCores, row-sharded):
  * K_bl [N, S-slice] built once on-device in fp16 (fp8 fails the accuracy
    budget: the mean-field dynamics amplify ~3% kernel noise into ~3e-2
    worst-pixel errors) and kept SBUF-resident per core.
  * The K_bl build (d2 matmul + Exp on the scalar engine) is fused with
    iteration-0's bilateral accumulation: chunk matmuls fire as their K_bl
    columns come out of the exp, hiding all of iteration 0's PE work inside
    the ACT-bound exp window.  Optionally a small fraction of exp windows
    run on the vector engine as a 16-bit Schraudolph (u16 bitcast fp16).
  * q carries a 22nd norm channel = 0.125 and labels pre-scaled by 1.25, so
    P_l / P_norm = 10 * message (W_BILATERAL folded); the spatial blur
    matrices absorb the 1/1.25.
  * Spatial kernel is separable: x-blur per core pre-gather, y-blur
    post-gather (W_SPATIAL/norm folded into host blur matrices); the
    x-blurred t1 rides the AllGather in fp8 (accuracy-checked).
  * Per-iteration tail is group-pipelined (3 y-groups: burst -> transpose ->
    softmax -> x-blur while the next group's burst runs), q is repacked to
    chunk-major via PE transposes + a repack matmul (no partition-shift DMA
    dance), and the gathered payloads are reassembled with one merged DMA
    per destination tensor.
"""
import sys
sys.path.insert(0, "/opt/trn_rl_repo")
import os
import numpy as np
import ml_dtypes

H = W = 96
N = H * W                  # 9216
L = 21
LE = L + 1                 # 22 channels (21 labels + norm channel)
ALPHA, BETA, GAMMA = 80.0, 13.0, 3.0
W_SPATIAL, W_BILATERAL = 3.0, 10.0
NUM_ITERATIONS = 5
NCORES = 8
S = N // NCORES            # 1152 rows per core
YPC = H // NCORES          # 12 image rows per core
CH = N // 128              # 72 chunks of 128 rows (global j)
KCOLS = CH * S             # 82944 K_bl columns (fp16: 2B each)
WIN = 1024                 # d2/exp window (81 windows exactly)
NWIN = KCOLS // WIN
PAYQ = 128 * 9 * LE * 2    # 50688 payload bytes (q part, fp16)
PAYT = S * LE              # 25344 payload bytes (t1 part, fp8)
A16 = 1024.0 / float(np.log(2.0))   # fp16 Schraudolph scale (10 mantissa bits)
B16 = float(15 * 1024)              # fp16 exponent bias 15 << 10
ONESV = 0.125              # q norm-channel value
QSCALE = 1.25              # label-channel pre-scale; 1.25/0.125 = W_BILATERAL
DVE_SHARE = float(os.environ.get("CRF_DVE_SHARE", "0.25"))

# Bilateral output groups (by y-rows of this core's slice) for tail pipelining
GROUPS = (
    (0, 5, ((0, 480),)),
    (5, 10, ((480, 32), (512, 448))),
    (10, 12, ((960, 64), (1024, 128))),
)
T2_SLICES = [(0, 512), (512, 512), (1024, 512), (1536, 512), (2048, 64)]

LAST_EXEC_NS = None
_CACHE = {}


def _dve_window(w):
    """Deterministic ACT/DVE interleave for exp windows."""
    return int((w + 1) * DVE_SHARE) != int(w * DVE_SHARE)


def _build_bass(sim1=False):
    """Build the kernel. sim1=True builds a single-core variant where the
    AllGather is replaced by a broadcast DRAM copy (for TimelineSim)."""
    key = "nc_sim1" if sim1 else "nc"
    if key in _CACHE:
        return _CACHE[key]
    import concourse.bass as bass  # noqa: F401
    from concourse import bacc
    import concourse.mybir as mybir
    import concourse.tile as tile

    f32 = mybir.dt.float32
    bf16 = mybir.dt.bfloat16
    fp16 = mybir.dt.float16
    f8 = mybir.dt.float8e4
    u16 = mybir.dt.uint16
    u8 = mybir.dt.uint8
    AF = mybir.ActivationFunctionType
    OP = mybir.AluOpType
    AX = mybir.AxisListType

    nc = bacc.Bacc("TRN2", target_bir_lowering=False, debug=False,
                   num_devices=1 if sim1 else NCORES)

    featL_d = nc.dram_tensor("featL", [21, N], bf16, kind="ExternalInput")
    featR_d = nc.dram_tensor("featR", [21, S], bf16, kind="ExternalInput")
    uxy_d = nc.dram_tensor("uxy", [W, YPC * LE], f32, kind="ExternalInput")
    Ax_d = nc.dram_tensor("Ax", [W, W], bf16, kind="ExternalInput")
    Ay_d = nc.dram_tensor("Ay", [H, YPC], bf16, kind="ExternalInput")
    qsb0_d = nc.dram_tensor("qsb0", [128, CH * LE], fp16, kind="ExternalInput")
    t1f0_d = nc.dram_tensor("t1f0", [H, LE * W], f8, kind="ExternalInput")
    id22_d = nc.dram_tensor("id22", [LE, LE], f32, kind="ExternalInput")
    id96_d = nc.dram_tensor("id96", [W, W], f32, kind="ExternalInput")
    i22h_d = nc.dram_tensor("i22h", [LE, LE], fp16, kind="ExternalInput")
    qout_d = nc.dram_tensor("qout", [S, L], f32, kind="ExternalOutput")
    dbgsel = int(os.environ.get("CRF_DEBUG", "0"))
    dbg = dbgsel > 0
    if dbg:
        dbg_kbl = nc.dram_tensor("dbg_kbl", [128, 4 * S], fp16,
                                 kind="ExternalOutput")
        dbg_pbl = nc.dram_tensor("dbg_pbl", [LE, S], f32, kind="ExternalOutput")
        dbg_t2r = nc.dram_tensor("dbg_t2r", [LE, YPC * W], f32,
                                 kind="ExternalOutput")
        dbg_qy = nc.dram_tensor("dbg_qy", [W, YPC * LE], f32,
                                kind="ExternalOutput")
        dbg_qsb1 = nc.dram_tensor("dbg_qsb1", [128, CH * LE], fp16,
                                  kind="ExternalOutput")
        dbg_t1f1 = nc.dram_tensor("dbg_t1f1", [H, LE * W], f8,
                                  kind="ExternalOutput")

    with tile.TileContext(nc) as tc:
        with (
            tc.tile_pool(name="const", bufs=1) as constp,
            tc.tile_pool(name="kbl", bufs=1) as kblp,
            tc.tile_pool(name="work", bufs=1) as work,
            tc.tile_pool(name="dram", bufs=2, space="DRAM") as dram,
        ):
            Ax = constp.tile([W, W], bf16)
            nc.sync.dma_start(Ax[:], Ax_d[:])
            Ay = constp.tile([H, YPC], bf16)
            nc.sync.dma_start(Ay[:], Ay_d[:])
            uxy = constp.tile([W, YPC * LE], f32)
            nc.sync.dma_start(uxy[:], uxy_d[:])
            uxy3 = uxy[:].rearrange("x (y l) -> x y l", y=YPC, l=LE)
            id22 = constp.tile([LE, LE], f32)
            nc.sync.dma_start(id22[:], id22_d[:])
            id96 = constp.tile([W, W], f32)
            nc.sync.dma_start(id96[:], id96_d[:])
            i22h = constp.tile([LE, LE], fp16)
            nc.sync.dma_start(i22h[:], i22h_d[:])
            featR = constp.tile([21, S], bf16)
            nc.sync.dma_start(featR[:], featR_d[:])
            Kbl = kblp.tile([128, KCOLS], fp16)
            Kbl_u16 = Kbl[:].bitcast(u16)

            qsb = work.tile([128, CH * LE], fp16, tag="qsb")
            t1full = work.tile([H, LE * W], f8, tag="t1full")
            nc.sync.dma_start(qsb[:], qsb0_d[:])
            nc.scalar.dma_start(t1full[:], t1f0_d[:])

            # ------- precompute K_bl (deep-pipelined over all 8 banks) ------
            with (
                tc.tile_pool(name="pre_sb", bufs=2) as pre_sb,
                tc.tile_pool(name="pre_ps", bufs=4, space="PSUM") as pre_ps,
            ):
                flb, flb_idx = None, -1
                for wi in range(NWIN):
                    X = wi * WIN
                    d2 = pre_ps.tile([128, WIN], f32, tag="d2")
                    cuts = sorted({X, X + WIN}
                                  | set(range((X // 512 + 1) * 512,
                                              X + WIN, 512))
                                  | set(range((X // S + 1) * S, X + WIN, S)))
                    for a, b in zip(cuts[:-1], cuts[1:]):
                        ch = a // S
                        if ch // 8 != flb_idx:
                            flb_idx = ch // 8
                            flb = pre_sb.tile([21, 1024], bf16, tag="fl")
                            nc.sync.dma_start(
                                flb[:],
                                featL_d[:, flb_idx * 1024:(flb_idx + 1) * 1024])
                        ci = ch - flb_idx * 8
                        nc.tensor.matmul(d2[:, a - X: b - X],
                                         flb[:, ci * 128:(ci + 1) * 128],
                                         featR[:, a - ch * S: b - ch * S],
                                         start=True, stop=True)
                    if _dve_window(wi):
                        # fp16 Schraudolph: u16 = sat(rne(max(d2s + B16, 0)));
                        # its piecewise-linear decode error (~3% rel) is only
                        # acceptable on a small fraction of windows.
                        nc.vector.tensor_scalar(Kbl_u16[:, X:X + WIN],
                                                d2[:], B16, 0.0,
                                                OP.add, OP.max)
                    else:
                        nc.scalar.activation(Kbl[:, X:X + WIN], d2[:],
                                             AF.Exp, scale=float(1.0 / A16))


            if dbg and dbgsel in (1, 3):
                nc.sync.dma_start(dbg_kbl.ap(), Kbl[:, 0:4 * S])

            # ---------- iterations -----------------------------------------
            # psA: persistent bilateral accumulator (first => 2048-aligned)
            psA_ctx = tc.tile_pool(name="psA", bufs=1, space="PSUM")
            psA = psA_ctx.__enter__()
            P_bl = psA.tile([LE, S], f32, tag="A")

            def chunk_matmuls(c, pieces, qsb_t):
                lhsT = qsb_t[:, c * LE:(c + 1) * LE]
                for (o, w) in pieces:
                    nc.tensor.matmul(P_bl[:, o:o + w], lhsT,
                                     Kbl[:, c * S + o:c * S + o + w],
                                     start=(c == 0), stop=(c == CH - 1))

            psum_ctx = tc.tile_pool(name="psum", bufs=1, space="PSUM")
            psum = psum_ctx.__enter__()
            # reserve the 2048B-aligned matmul-target slots first
            psum.tile([128, 512], f32, tag="mm512", bufs=2, name="mm512r")
            psum.tile([YPC, 512], f32, tag="yb", bufs=2, name="ybr")

            def yblur_t2r(t2r, t1full_t):
                """t2 = Ay^T @ t1full -> tb; DRAM-roundtrip transpose to
                [LE, YPC*W] (unaries are added later, in x-major)."""
                tb = work.tile([YPC, LE * W], f32, tag="t2b")
                t2scr = dram.tile([YPC, LE * W], f32, tag="t2scr")
                for pi, (o, w) in enumerate(T2_SLICES):
                    t2p = psum.tile([YPC, 512], f32, tag="yb", bufs=2)
                    nc.tensor.matmul(t2p[:, 0:w], Ay[:], t1full_t[:, o:o + w],
                                     start=True, stop=True)
                    if pi % 2:
                        nc.vector.tensor_copy(tb[:, o:o + w], t2p[:, 0:w])
                    else:
                        nc.scalar.copy(tb[:, o:o + w], t2p[:, 0:w])
                nc.scalar.dma_start(t2scr[:], tb[:])
                nc.sync.dma_start(
                    t2r[:].rearrange("l (y x) -> l y x", y=YPC, x=W),
                    t2scr[:].rearrange("y (l x) -> l y x", l=LE, x=W),
                )

            qag_prev = None
            for it in range(NUM_ITERATIONS):
                last = it == NUM_ITERATIONS - 1
                if it > 0:
                    qagQ, qagT = qag_prev
                    qsb = work.tile([128, CH * LE], fp16, tag="qsb")
                    t1full = work.tile([H, LE * W], f8, tag="t1full")
                    # merged reassembly: one DMA per destination tensor
                    nc.sync.dma_start(
                        qsb[:].bitcast(u8)
                        .rearrange("p (r c) -> p r c", r=NCORES),
                        qagQ[:].rearrange("r (p c) -> p r c", p=128))
                    # t1 payload is y-major so (r, y) fuses into the partition
                    # dim with a uniform stride: one DMA for all 8 cores.
                    nc.scalar.dma_start(
                        t1full[:],
                        qagT[:].rearrange("r (y c) -> (r y) c", y=YPC))
                if dbg and it == 1 and dbgsel in (1, 2):
                    nc.sync.dma_start(dbg_qsb1.ap(), qsb[:])
                    nc.sync.dma_start(dbg_t1f1.ap(), t1full[:])
                t2r = work.tile([LE, YPC * W], f32, tag="t2r")
                yblur_t2r(t2r, t1full[:])

                # ------ per-group burst (iters >= 1), then its combine /
                # softmax tail so it pipelines behind the next group's burst --
                pbs = work.tile([LE, S], f32, tag="pbs")
                qy = work.tile([W, YPC * LE], f32, tag="qy")
                qy3 = qy[:].rearrange("x (y l) -> x y l", y=YPC, l=LE)
                ssum = work.tile([W, YPC], f32, tag="ssum")
                rec12 = work.tile([W, YPC], f32, tag="rec12")
                blsc = work.tile([W, YPC * LE], f32, tag="blsc")
                blsc3 = blsc[:].rearrange("x (y l) -> x y l", y=YPC, l=LE)
                if not last:
                    qyb = work.tile([W, YPC * LE], fp16, tag="qyb")
                    t1Ts = work.tile([LE, S], f8, tag="t1Ts")
                    ql16 = work.tile([LE, S], fp16, tag="ql16")
                for gi, (y0, y1, pieces) in enumerate(GROUPS):
                    for c in range(CH):
                        chunk_matmuls(c, pieces, qsb[:])
                    c0, c1 = y0 * W, y1 * W
                    ng = y1 - y0
                    nc.scalar.copy(pbs[:, c0:c1], P_bl[:, c0:c1])
                    tp = psum.tile([W, 2 * 5 * LE], f32, tag="tp", bufs=1)
                    for k, y in enumerate(range(y0, y1)):
                        nc.tensor.transpose(
                            tp[:, (5 + k) * LE:(5 + k + 1) * LE],
                            pbs[:, y * W:(y + 1) * W], id22[:])
                        nc.tensor.transpose(tp[:, k * LE:(k + 1) * LE],
                                            t2r[:, y * W:(y + 1) * W], id22[:])
                    tp0 = tp[:, 0:ng * LE].rearrange("x (y l) -> x y l", l=LE)
                    tpB = tp[:, 5 * LE:(5 + ng) * LE].rearrange(
                        "x (y l) -> x y l", l=LE)
                    nc.vector.reciprocal(rec12[:, y0:y1][:, :, None],
                                         tpB[:, :, L:LE])
                    nc.vector.tensor_tensor(
                        blsc3[:, y0:y1], tpB,
                        rec12[:, y0:y1][:, :, None].to_broadcast([W, ng, LE]),
                        OP.mult)
                    nc.vector.tensor_tensor(tp0, tp0, uxy3[:, y0:y1], OP.add)
                    nc.vector.tensor_tensor(tp0, tp0, blsc3[:, y0:y1], OP.add)
                    nc.scalar.activation(qy[:, y0 * LE:y1 * LE],
                                         tp[:, 0:ng * LE], AF.Exp)
                    nc.vector.reduce_sum(ssum[:, y0:y1],
                                         qy3[:, y0:y1, 0:L], axis=AX.X)
                    nc.vector.reciprocal(ssum[:, y0:y1], ssum[:, y0:y1])
                    nc.vector.tensor_tensor(
                        qy3[:, y0:y1, 0:L], qy3[:, y0:y1, 0:L],
                        ssum[:, y0:y1][:, :, None].to_broadcast([W, ng, L]),
                        OP.mult)
                    if last:
                        nc.sync.dma_start(
                            qout_d.ap()[y0 * W:y1 * W, :]
                                 .rearrange("(y x) l -> x y l", x=W),
                            qy3[:, y0:y1, 0:L])
                        continue
                    nc.vector.memset(qy3[:, y0:y1, L:LE], ONESV / QSCALE)
                    nc.vector.tensor_scalar(qyb[:, y0 * LE:y1 * LE],
                                            qy[:, y0 * LE:y1 * LE],
                                            QSCALE, None, OP.mult)
                    # x-blur this group's rows into one psum bank
                    xb = psum.tile([128, 512], f32, tag="mm512", bufs=2,
                                   name="mm512r")
                    for k, y in enumerate(range(y0, y1)):
                        nc.tensor.matmul(xb[0:LE, k * W:(k + 1) * W],
                                         qyb[:, y * LE:(y + 1) * LE], Ax[:],
                                         start=True, stop=True)
                    nc.scalar.copy(t1Ts[:, c0:c1], xb[0:LE, 0:(c1 - c0)])
                    # transpose q to label-major for the chunk repack
                    qlt = psum.tile([128, 512], f32, tag="mm512", bufs=2,
                                    name="mm512r")
                    for k, y in enumerate(range(y0, y1)):
                        nc.tensor.transpose(qlt[0:LE, k * W:(k + 1) * W],
                                            qy[:, y * LE:(y + 1) * LE],
                                            id96[:])
                    nc.scalar.activation(ql16[:, c0:c1],
                                         qlt[0:LE, 0:(c1 - c0)],
                                         AF.Copy, scale=QSCALE)

                if dbg and it == 0 and dbgsel in (1, 3):
                    nc.sync.dma_start(dbg_pbl.ap(), pbs[:])
                    nc.sync.dma_start(dbg_t2r.ap(), t2r[:])
                    nc.sync.dma_start(dbg_qy.ap(), qy[:])
                if last:
                    continue

                # chunk-major repack: q128[p, c, :] = ql16[:, c*128:..]^T
                q128 = work.tile([128, 9 * LE], fp16, tag="q128")
                q128ps = psum.tile([128, 512], f32, tag="mm512", bufs=2,
                                   name="mm512r")
                for c in range(9):
                    nc.tensor.matmul(q128ps[:, c * LE:(c + 1) * LE],
                                     ql16[:, c * 128:(c + 1) * 128], i22h[:],
                                     start=True, stop=True)
                nc.scalar.copy(q128[:], q128ps[:, 0:9 * LE])

                # payload staging (t1 part re-laid y-major) + AllGather
                plQ = dram.tile([1, PAYQ], u8, tag="plQ")
                plT = dram.tile([1, PAYT], f8, tag="plT")
                nc.sync.dma_start(
                    plQ[0:1, :].rearrange("a (p c) -> (a p) c", p=128),
                    q128[:].bitcast(u8))
                nc.scalar.dma_start(
                    plT[0:1, :].rearrange("a (y l x) -> (a l) y x",
                                          y=YPC, l=LE),
                    t1Ts[:].rearrange("l (y x) -> l y x", y=YPC))
                qagQ = dram.tile([NCORES, PAYQ], u8, tag="qagQ")
                qagT = dram.tile([NCORES, PAYT], f8, tag="qagT")
                if sim1:
                    nc.sync.dma_start(
                        qagQ[:], plQ[0:1, :].to_broadcast([NCORES, PAYQ]))
                    nc.scalar.dma_start(
                        qagT[:], plT[0:1, :].to_broadcast([NCORES, PAYT]))
                else:
                    nc.gpsimd.collective_compute(
                        "AllGather", OP.bypass,
                        replica_groups=[list(range(NCORES))],
                        ins=[plQ.opt()], outs=[qagQ.opt()])
                    nc.gpsimd.collective_compute(
                        "AllGather", OP.bypass,
                        replica_groups=[list(range(NCORES))],
                        ins=[plT.opt()], outs=[qagT.opt()])
                qag_prev = (qagQ, qagT)
            psum_ctx.__exit__(None, None, None)
            psA_ctx.__exit__(None, None, None)

    nc.compile()
    _CACHE[key] = nc
    return nc


def _host_prepare(unaries, rgb):
    u = np.asarray(unaries, np.float32).reshape(N, L)
    c = np.asarray(rgb, np.float32).reshape(N, 3)
    bfd = ml_dtypes.bfloat16
    f8d = ml_dtypes.float8_e4m3
    f16 = np.float16

    ys, xs = np.meshgrid(np.arange(H, dtype=np.float64),
                         np.arange(W, dtype=np.float64), indexing="ij")
    pos = np.stack([ys.ravel(), xs.ravel()], -1)            # [N, 2]
    g = np.concatenate([c.astype(np.float64) / BETA, pos / ALPHA], 1)
    g = g - g.mean(0, keepdims=True)
    sq = (g * g).sum(1)
    ones = np.ones(N, np.float64)
    L7 = np.concatenate([g.T, ones[None], (-0.5 * sq)[None]], 0)         # [7,N]
    R7 = np.concatenate([g.T, (-0.5 * sq)[None], ones[None]], 0) * A16   # [7,N]
    Lhi = L7.astype(bfd)
    Llo = (L7 - Lhi.astype(np.float64)).astype(bfd)
    Rhi = R7.astype(bfd)
    Rlo = (R7 - Rhi.astype(np.float64)).astype(bfd)
    # dot = Lhi.Rhi + Lhi.Rlo + Llo.Rhi  (Llo.Rlo dropped, ~1e-3)
    featL = np.ascontiguousarray(np.concatenate([Lhi, Lhi, Llo], 0))  # [21,N]
    featR = np.ascontiguousarray(np.concatenate([Rhi, Rlo, Rhi], 0))  # [21,N]

    d = np.arange(W, dtype=np.float64)
    A = np.exp(-(d[:, None] - d[None, :]) ** 2 / (2.0 * GAMMA * GAMMA))
    nvec = A.sum(0)
    Ax = np.ascontiguousarray((A / nvec[None, :]).astype(bfd))

    um = u.max(1, keepdims=True)
    e = np.exp(u - um)
    q0 = e / e.sum(1, keepdims=True)
    q0s = np.concatenate([QSCALE * q0, np.full((N, 1), ONESV, np.float64)],
                         1)                                   # [N, 22] scaled
    qsb0 = np.ascontiguousarray(
        q0s.reshape(CH, 128, LE).transpose(1, 0, 2)
        .reshape(128, CH * LE)).astype(f16)

    q3 = q0s.reshape(H, W, LE)
    t1 = np.einsum("Xx,yXl->ylx", A / nvec[None, :], q3)      # [96, 22, 96]
    t1f0 = np.ascontiguousarray(t1.reshape(H, LE * W).astype(f8d))

    id22 = np.eye(LE, dtype=np.float32)
    id96 = np.eye(W, dtype=np.float32)
    i22h = np.eye(LE, dtype=np.float32).astype(f16)

    in_maps = []
    for core in range(NCORES):
        rows = slice(core * S, (core + 1) * S)
        ue = np.full((S, LE), -50.0, np.float32)
        ue[:, 0:L] = u[rows]
        # x-major unaries: uxy[x, y*LE + l] = ue[y*W + x, l]
        uxy_c = np.ascontiguousarray(
            ue.reshape(YPC, W, LE).transpose(1, 0, 2).reshape(W, YPC * LE))
        yc = slice(core * YPC, (core + 1) * YPC)
        Ay_c = np.ascontiguousarray(
            (A[:, yc] * (W_SPATIAL / QSCALE / nvec[yc])[None, :]).astype(bfd))
        in_maps.append({
            "featL": featL,
            "featR": np.ascontiguousarray(featR[:, rows]),
            "uxy": uxy_c,
            "Ax": Ax,
            "Ay": Ay_c,
            "qsb0": qsb0,
            "t1f0": t1f0,
            "id22": id22,
            "id96": id96,
            "i22h": i22h,
        })
    return in_maps


def _get_runner():
    """Compile once; return (fn, in_names, out_names) where fn maps
    concatenated global numpy inputs -> list of per-core output dicts."""
    if "runner" in _CACHE:
        return _CACHE["runner"]
    import jax
    from jax.sharding import Mesh, PartitionSpec
    from jax.experimental.shard_map import shard_map
    import concourse.mybir as mybir
    from concourse import bass2jax

    nc = _build_bass()
    bass2jax.install_neuronx_cc_hook()

    partition_name = (nc.partition_id_tensor.name
                      if nc.partition_id_tensor else None)
    in_names, out_names, out_avals, zero_outs = [], [], [], []
    for alloc in nc.m.functions[0].allocations:
        if not isinstance(alloc, mybir.MemoryLocationSet):
            continue
        name = alloc.memorylocations[0].name
        if alloc.kind == "ExternalInput":
            if name != partition_name:
                in_names.append(name)
        elif alloc.kind == "ExternalOutput":
            shape = tuple(alloc.tensor_shape)
            dtype = mybir.dt.np(alloc.dtype)
            out_names.append(name)
            out_avals.append(jax.core.ShapedArray(shape, dtype))
            zero_outs.append(np.zeros(shape, dtype))
    n_params = len(in_names)
    all_in_names = list(in_names) + list(out_names)
    if partition_name is not None:
        all_in_names.append(partition_name)

    def _body(*args):
        operands = list(args)
        if partition_name is not None:
            operands.append(bass2jax.partition_id_tensor())
        outs = bass2jax._bass_exec_p.bind(
            *operands,
            out_avals=tuple(out_avals),
            in_names=tuple(all_in_names),
            out_names=tuple(out_names),
            lowering_input_output_aliases=(),
            sim_require_finite=False,
            sim_require_nnan=False,
            nc=nc,
        )
        return tuple(outs)

    devices = jax.devices()[:NCORES]
    mesh = Mesh(np.asarray(devices), ("core",))
    n_outs = len(out_names)
    in_specs = (PartitionSpec("core"),) * (n_params + n_outs)
    out_specs = (PartitionSpec("core"),) * n_outs
    donate = tuple(range(n_params, n_params + n_outs))
    fn = jax.jit(
        shard_map(_body, mesh=mesh, in_specs=in_specs, out_specs=out_specs,
                  check_rep=False),
        donate_argnums=donate, keep_unused=True)
    _CACHE["runner"] = (fn, in_names, out_names, out_avals, zero_outs)
    return _CACHE["runner"]


def _concat_inputs(in_maps, in_names):
    return [np.concatenate([np.asarray(in_maps[c][nm]) for c in range(NCORES)],
                           axis=0) for nm in in_names]


def _run(in_maps):
    fn, in_names, out_names, out_avals, zero_outs = _get_runner()
    concat_in = _concat_inputs(in_maps, in_names)
    concat_zeros = [np.zeros((NCORES * z.shape[0], *z.shape[1:]), z.dtype)
                    for z in zero_outs]
    out_arrs = fn(*concat_in, *concat_zeros)
    return out_arrs, out_names, out_avals


def kernel(unaries, rgb):
    in_maps = _host_prepare(unaries, rgb)
    out_arrs, out_names, out_avals = _run(in_maps)
    qi = out_names.index("qout")
    q = np.asarray(out_arrs[qi]).reshape(NCORES, S, L).reshape(N, L)
    return np.ascontiguousarray(q[None].astype(np.float32))


def time_kernel(unaries, rgb, iters=20):
    """Steady-state per-call wall time of the compiled 8-core executable,
    with inputs pre-staged on device."""
    import time as _time
    import jax
    in_maps = _host_prepare(unaries, rgb)
    fn, in_names, out_names, out_avals, zero_outs = _get_runner()
    concat_in = _concat_inputs(in_maps, in_names)

    def once():
        concat_zeros = [np.zeros((NCORES * z.shape[0], *z.shape[1:]), z.dtype)
                        for z in zero_outs]
        outs = fn(*concat_in, *concat_zeros)
        jax.block_until_ready(outs)
        return outs

    once()  # warm
    times = []
    for _ in range(iters):
        t0 = _time.perf_counter()
        once()
        times.append(_time.perf_counter() - t0)
    return min(times), sorted(times)[len(times) // 2]
